# revision 22
# baseline (speedup 1.0000x reference)
"""NeuralSDE forecasting kernel for 8x Trainium2 NeuronCores (Bass/Tile).

Data-parallel over batch B=256 across 8 cores (32 batch elems per core).
The per-core scan runs feature-major ("transposed"): state y.T lives in a
[128 partitions, 4*32] SBUF tile; column block k holds features
128k..128k+128 of the 32 local batch columns. Orientation: out = lhsT.T @
rhs with weight tiles stationary and the state as the 32-col moving
operand. No transposes anywhere.

Precision: the 255-step recurrence amplifies per-step rounding noise
~100x, so bf16 operands fail (0.19 rel err) and fp32 matmuls are
weight-load bound. Each weight is split W = W_hi + W_lo (both bf16) and
each state operand y into y_hi + y_lo; the product uses three bf16
matmuls (y_hi@W_hi + y_lo@W_hi + y_hi@W_lo, fp32 PSUM accumulate) which
restores ~fp32 accuracy (8.6e-4 measured end-to-end) while loading
weights via the 2x Fast-Weight-Load bf16 path.

Perf structure (v2): the scan is LDWEIGHTS-bound (~43ns per 128x128 bf16
weight tile, 96 tiles/step). Everything else is arranged to keep the PE
weight-load pipeline saturated:
- The hi/lo halves of every N=64 matmul write the SAME psum columns via
  a stride-0 (broadcast) output AP: PSUM's per-element has_written bit
  makes the second half accumulate onto the first, so no DVE fold ops
  exist at all. ACT reads pre-activations straight from PSUM.
- The control projection u_t = x~_t @ [W1x; b1] is computed in-step by
  streaming the resident x~ tile into the psA accumulation group (2
  matmuls per m-chunk: [x_hi|x_lo]@Wb_hi stride-0 + x_hi@Wb_lo). These
  matmuls have no step dependencies, so they both kill the old
  precompute phase/DMA gather and give the scheduler always-ready PE
  work to fill the inter-step dependency gap.
- b2 enters psB as rank-1 matmuls (bias row x ones), bg rides the tau
  activation's per-partition bias (4 [128,32] ACT ops, one per m-chunk).
- sigmoid(x) = 0.5*(1+tanh(x/2)) keeps the scan on the Tanh ACT table;
  the 0.5 factors are folded into the host-prescaled dW.
- Remaining DVE work is 6 ops/step (h_lo split, t1, yh2, and the
  3-op y update producing the bf16 hi/lo stream + f32 master). The hi
  half is produced first so the next step's W_lo matmuls (which need
  only y_hi) start one DVE op after the f tanh.
"""

import os
import sys

sys.path.insert(0, "/opt/trn_rl_repo")

import numpy as np
import ml_dtypes

import concourse.bass as bass
import concourse.bacc as bacc
import concourse.mybir as mybir
import concourse.tile as tile
from concourse.bass_utils import run_bass_kernel_spmd

B, T, C, H, O = 256, 256, 32, 512, 32
OUT_TIME = 32
NCORES = 8
BL = B // NCORES  # 32 batch elements per core
NT = int(os.environ.get("BASS_NT", T - 1))  # 255 scan steps
SAVE0 = NT - OUT_TIME  # first step whose y_next lands in the output tail
KC = H // 128  # 4 feature chunks
F32 = mybir.dt.float32
BF16 = mybir.dt.bfloat16
BF = ml_dtypes.bfloat16

Tanh = mybir.ActivationFunctionType.Tanh
Relu = mybir.ActivationFunctionType.Relu
Identity = mybir.ActivationFunctionType.Identity

_BUILT = None


def _build_nc():
    nc = bacc.Bacc("TRN2", target_bir_lowering=False, debug=False)

    # --- DRAM I/O (per-core shards; weights replicated) ---
    # x~ for all t, feature-major, packed [hi(32)|lo(32)] per timestep
    NTP = T  # 256 t-slots (255 used, 1 zero pad)
    d_xhl = nc.dram_tensor("xhl", [C + 1, NTP * 2 * BL], BF16, kind="ExternalInput")
    d_x0 = nc.dram_tensor("x0", [C + 1, BL], F32, kind="ExternalInput")
    d_dw = nc.dram_tensor("dw", [NT, 128, KC * BL], F32, kind="ExternalInput")
    wnames = ["w1y", "w2", "wg"]
    d_w = {
        (n, p): nc.dram_tensor(f"{n}_{p}", [128, KC * H], BF16, kind="ExternalInput")
        for n in wnames
        for p in ("hi", "lo")
    }
    d_w1b = {
        p: nc.dram_tensor(f"w1b_{p}", [C + 1, H], BF16, kind="ExternalInput")
        for p in ("hi", "lo")
    }
    d_b2r = nc.dram_tensor("b2r", [2, H], BF16, kind="ExternalInput")
    d_wini = nc.dram_tensor("wini", [C + 1, H], F32, kind="ExternalInput")
    d_bgb = nc.dram_tensor("bgb", [128, KC * BL], F32, kind="ExternalInput")
    d_wh1 = nc.dram_tensor("wh1", [128, KC * H], F32, kind="ExternalInput")
    d_wh2 = nc.dram_tensor("wh2", [128, KC * O], F32, kind="ExternalInput")
    d_bh1 = nc.dram_tensor("bh1t", [128, KC], F32, kind="ExternalInput")
    d_bh2 = nc.dram_tensor("bh2t", [O, 1], F32, kind="ExternalInput")
    d_out = nc.dram_tensor("out", [O, OUT_TIME * BL], F32, kind="ExternalOutput")

    with tile.TileContext(nc) as tc:
        with (
            tc.tile_pool(name="const", bufs=1) as const,
            tc.tile_pool(name="dwp", bufs=6) as dwp,
            tc.tile_pool(name="yp", bufs=2) as yp,
            tc.tile_pool(name="tmp", bufs=3) as tmp,
            tc.tile_pool(name="pp", bufs=1, space="PSUM") as pp,
            tc.tile_pool(name="pc", bufs=2, space="PSUM") as pc,
            tc.tile_pool(name="ph", bufs=1, space="PSUM") as ph,
        ):
            # --- resident weights ---
            w_s = {}
            for key, d in d_w.items():
                w_s[key] = const.tile(
                    [128, KC * H], BF16, tag=f"{key[0]}_{key[1]}",
                    name=f"{key[0]}_{key[1]}_s",
                )
                nc.sync.dma_start(out=w_s[key][:], in_=d[:])
            w1b_s = {}
            for p, d in d_w1b.items():
                w1b_s[p] = const.tile([C + 1, H], BF16, tag=f"w1b{p}", name=f"w1b_{p}_s")
                nc.sync.dma_start(out=w1b_s[p][:], in_=d[:])
            b2r = const.tile([2, H], BF16, tag="b2r")
            nc.sync.dma_start(out=b2r[:], in_=d_b2r[:])
            wini = const.tile([C + 1, H], F32, tag="wini")
            bgb = const.tile([128, KC * BL], F32, tag="bgb")
            wh1 = const.tile([128, KC * H], F32, tag="wh1")
            wh2 = const.tile([128, KC * O], F32, tag="wh2")
            bh1 = const.tile([128, KC], F32, tag="bh1")
            bh2 = const.tile([O, 1], F32, tag="bh2")
            x0 = const.tile([C + 1, BL], F32, tag="x0")
            xhl = const.tile([C + 1, NTP * 2 * BL], BF16, tag="xhl")
            slab = const.tile([128, OUT_TIME * 128], F32, tag="slab")
            rT = const.tile([128, KC * 1024], F32, tag="rT")
            outs = const.tile([O, OUT_TIME * BL], F32, tag="outs")
            for dst, src in [
                (wini, d_wini), (bgb, d_bgb), (wh1, d_wh1), (wh2, d_wh2),
                (bh1, d_bh1), (bh2, d_bh2), (x0, d_x0), (xhl, d_xhl),
            ]:
                nc.sync.dma_start(out=dst[:], in_=src[:])
            # ones rows (partition 0) for the rank-1 bias matmuls: both
            # rows 1.0 so a [2,128] stationary sums b2_hi + b2_lo in one load
            ones_t = const.tile([2, BL], BF16, tag="ones")
            nc.vector.memset(ones_t[:], 1.0)
            ones2 = ones_t[:]

            def wsl(n, p, k, m):  # lhsT tile (k, m) of weight n, part p
                return w_s[(n, p)][:, k * H + m * 128 : k * H + (m + 1) * 128]

            def fold_out(ps, m):
                # stride-0 psum view: both halves of an N=64 matmul land on
                # the same 32 columns (2nd write accumulates via has_written)
                return ps[:, m * BL : (m + 1) * BL].unsqueeze(1).broadcast_to(
                    [128, 2, BL]
                )

            def hl_pairs(hl, k):
                # [v_hi_k | v_lo_k] as a strided N=64 moving operand
                return hl[:].rearrange("p (h q) -> p h q", h=2)[
                    :, :, k * BL : (k + 1) * BL
                ]

            # weight-product group: W_lo x v_hi first (depends only on the
            # hi half of the state, so it starts one DVE op after the
            # activation), then W_hi x [v_hi|v_lo] with stride-0 fold.
            def mm_group(ps, wname, vhl, first_start, last_stop):
                for m in range(KC):
                    for k in range(KC):
                        nc.tensor.matmul(
                            ps[:, m * BL : (m + 1) * BL],
                            wsl(wname, "lo", k, m),
                            vhl[:, k * BL : (k + 1) * BL],
                            start=first_start and (m == 0 and k == 0),
                            stop=False,
                        )
                for m in range(KC):
                    for k in range(KC):
                        nc.tensor.matmul(
                            fold_out(ps, m),
                            wsl(wname, "hi", k, m),
                            hl_pairs(vhl, k),
                            start=False,
                            stop=last_stop and (m == KC - 1 and k == KC - 1),
                        )

            # --- z0 (fp32, one-off) ---
            ps0 = pp.tile([128, 512], F32, tag="psA0", name="ps0")
            for m in range(KC):
                nc.tensor.matmul(
                    ps0[:, m * BL : (m + 1) * BL],
                    wini[:, m * 128 : (m + 1) * 128], x0[:],
                    start=(m == 0), stop=(m == KC - 1),
                )
            y_t = yp.tile([128, KC * BL], F32, tag="y")
            nc.vector.tensor_copy(y_t[:], ps0[:, 0:128])
            y = y_t[:]
            yhl = tmp.tile([128, KC * 2 * BL], BF16, tag="yhl", name="yhl_init")
            nc.vector.tensor_copy(yhl[:, 0:128], y)
            nc.vector.tensor_sub(yhl[:, 128:256], y, yhl[:, 0:128])

            # u_t = x~_t @ [W1x; b1] streamed in-step into psA from the
            # resident x~ tile; these matmuls have no step dependencies,
            # so they are emitted a step EARLY (see loop tail) to give the
            # PE ready work during the f/y-update boundary.
            def emit_xmms(psA, t):
                xs = slice(t * 2 * BL, (t + 1) * 2 * BL)
                for m in range(KC):
                    nc.tensor.matmul(
                        fold_out(psA, m),
                        w1b_s["hi"][:, m * 128 : (m + 1) * 128],
                        xhl[:, xs].rearrange("p (h b) -> p h b", h=2),
                        start=(m == 0), stop=False,
                    )
                for m in range(KC):
                    nc.tensor.matmul(
                        psA[:, m * BL : (m + 1) * BL],
                        w1b_s["lo"][:, m * 128 : (m + 1) * 128],
                        xhl[:, t * 2 * BL : t * 2 * BL + BL],
                        start=False, stop=False,
                    )

            def emit_rank1(psB):
                # b2 as rank-1 bias-rows x ones ([2,128] stationary sums
                # b2_hi + b2_lo in one load); no dependencies
                for m in range(KC):
                    nc.tensor.matmul(
                        psB[:, m * BL : (m + 1) * BL],
                        b2r[:, m * 128 : (m + 1) * 128],
                        ones2, start=(m == 0), stop=False,
                    )

            # --- scan ---
            # consecutive steps use alternating psum tags (bufs=1 each) so
            # step t+1's group-opening matmuls carry no WAR edge against
            # step t's still-pending ACT reads of the same tag
            psA = pp.tile([128, 512], F32, tag="psA1", name="psA_0")
            emit_xmms(psA, 0)
            psB = pp.tile([128, 512], F32, tag="psB1", name="psB_0")
            emit_rank1(psB)
            for t in range(NT):
                dw_t = dwp.tile([128, KC * BL], F32, tag="dw", name=f"dw_{t}")
                nc.sync.dma_start(out=dw_t[:], in_=d_dw[t])

                # psA = u_t + y@W1y
                mm_group(psA, "w1y", yhl, first_start=False, last_stop=True)

                # h = tanh(psA); the bf16-output instance IS h_hi (the ACT
                # spline is deterministic), so psB's W_lo matmuls start one
                # ACT op after the group; the f32 instance + lo subtract
                # run off the critical path.
                hhl = tmp.tile([128, KC * 2 * BL], BF16, tag="hhl", name=f"hhl_{t}")
                nc.scalar.activation(hhl[:, 0:128], psA[:, 0:128], Tanh)
                h = tmp.tile([128, KC * BL], F32, tag="h", name=f"h_{t}")
                nc.scalar.activation(h[:], psA[:, 0:128], Tanh)
                nc.vector.tensor_sub(hhl[:, 128:256], h[:], hhl[:, 0:128])

                # psC = y@Wg ; tau = tanh((psC + bg)/2): bg folded by one
                # DVE op (keeps the scalar queue free for the h/f tanhs),
                # then a single tanh ACT with scale 0.5.
                psC = pc.tile([128, 512], F32, tag="psC", name=f"psC_{t}")
                mm_group(psC, "wg", yhl, first_start=True, last_stop=True)
                preC = tmp.tile([128, KC * BL], F32, tag="preC", name=f"preC_{t}")
                nc.vector.tensor_add(preC[:], psC[:, 0:128], bgb[:])
                tau = tmp.tile([128, KC * BL], F32, tag="tau", name=f"tau_{t}")
                nc.scalar.activation(tau[:], preC[:], Tanh, scale=0.5)
                # t1 = (tau + 1) * dw ;  dw pre-scaled by 0.5*sqrt(dt)/dt
                t1 = tmp.tile([128, KC * BL], F32, tag="t1", name=f"t1_{t}")
                nc.vector.scalar_tensor_tensor(
                    t1[:], tau[:], 1.0, dw_t[:],
                    mybir.AluOpType.add, mybir.AluOpType.mult,
                )
                yh2 = tmp.tile([128, KC * BL], F32, tag="yh2", name=f"yh2_{t}")
                nc.vector.tensor_add(yh2[:], y, t1[:])

                # psB = b2 + h@W2 (b2 pre-emitted as rank-1 matmuls)
                mm_group(psB, "w2", hhl, first_start=False, last_stop=True)

                # pre-emit the next step's dependency-free matmuls so the
                # PE has work during the f-tanh / y-update chain
                psA_n = psB_n = None
                if t + 1 < NT:
                    par = (t + 1) % 2
                    psA_n = pp.tile([128, 512], F32, tag=f"psA{par}", name=f"psA_{t+1}")
                    emit_xmms(psA_n, t + 1)
                    psB_n = pp.tile([128, 512], F32, tag=f"psB{par}", name=f"psB_{t+1}")
                    emit_rank1(psB_n)

                f = tmp.tile([128, KC * BL], F32, tag="f", name=f"f_{t}")
                nc.scalar.activation(f[:], psB[:, 0:128], Tanh)

                # y_next = (y + t1) + f ; tail states land in the slab.
                # y_hi is produced first (bf16 add) so next-step W_lo
                # matmuls start one DVE op after tanh; the f32 master and
                # the lo residual follow off the critical path.
                if t >= SAVE0:
                    y2 = slab[:, (t - SAVE0) * 128 : (t - SAVE0 + 1) * 128]
                else:
                    y2_t = yp.tile([128, KC * BL], F32, tag="y", name=f"y_{t}")
                    y2 = y2_t[:]
                yhl = tmp.tile([128, KC * 2 * BL], BF16, tag="yhl", name=f"yhl_{t}")
                nc.vector.tensor_add(yhl[:, 0:128], yh2[:], f[:])
                nc.vector.tensor_add(y2, yh2[:], f[:])
                nc.vector.tensor_sub(yhl[:, 128:256], y2, yhl[:, 0:128])
                y = y2
                psA, psB = psA_n, psB_n

            # --- head (fp32): out = relu(z_tail@Wh1 + bh1) @ Wh2 + bh2 ---
            # slab columns: s*128 + k*32 + b  (s = tail step, k = feat chunk)
            slab_r = slab[:].rearrange(
                "p (s k b) -> p s k b", s=OUT_TIME, k=KC, b=BL
            )
            for m in range(KC):
                for hf in range(2):
                    ps1 = ph.tile([128, 512], F32, tag="psH", name=f"ps1_{m}_{hf}")
                    for k in range(KC):
                        nc.tensor.matmul(
                            ps1[:],
                            wh1[:, k * H + m * 128 : k * H + (m + 1) * 128],
                            slab_r[:, hf * 16 : (hf + 1) * 16, k, :],
                            start=(k == 0), stop=(k == KC - 1),
                        )
                    nc.scalar.activation(
                        rT[:, m * 1024 + hf * 512 : m * 1024 + (hf + 1) * 512],
                        ps1[:], Relu, bias=bh1[:, m : m + 1],
                    )
            for hf in range(2):
                ps2 = ph.tile([O, 512], F32, tag="psH2", name=f"ps2_{hf}")
                for m in range(KC):
                    nc.tensor.matmul(
                        ps2[:],
                        wh2[:, m * O : (m + 1) * O],
                        rT[:, m * 1024 + hf * 512 : m * 1024 + (hf + 1) * 512],
                        start=(m == 0), stop=(m == KC - 1),
                    )
                nc.scalar.activation(
                    outs[:, hf * 512 : (hf + 1) * 512], ps2[:], Identity,
                    bias=bh2[:],
                )
            nc.sync.dma_start(out=d_out[:], in_=outs[:])

    nc.compile()
    return nc


def _split(w):
    hi = np.asarray(w, BF)
    lo = (np.asarray(w, np.float32) - hi.astype(np.float32)).astype(BF)
    return hi, lo


def _prep_inputs(times, coeffs, final_index, dW, W_init, b_init, W1, b1, W2,
                 b2, Wg, bg, Wh1, bh1, Wh2, bh2):
    f32 = np.float32
    times = np.asarray(times, f32)
    dt = f32(max(np.min(times[1:] - times[:-1]), f32(0.001)))
    sq = f32(np.sqrt(dt))

    def lhsT_layout(w):  # [H, H] -> [128, KC*H] with (k,m) tile at k*H+m*128
        return np.ascontiguousarray(
            np.asarray(w, f32).reshape(KC, 128, H).transpose(1, 0, 2).reshape(128, KC * H)
        )

    W1 = np.asarray(W1, f32)
    shared = {}
    for name, w in [("w1y", dt * W1[:H]), ("w2", np.asarray(W2, f32)),
                    ("wg", dt * np.asarray(Wg, f32))]:
        hi, lo = _split(lhsT_layout(w))
        shared[f"{name}_hi"] = hi
        shared[f"{name}_lo"] = lo
    w1b = np.vstack([W1[H:], np.asarray(b1, f32)[None, :]])
    shared["w1b_hi"], shared["w1b_lo"] = _split(w1b)
    b2hi, b2lo = _split(np.asarray(b2, f32)[None, :])
    shared["b2r"] = np.ascontiguousarray(np.vstack([b2hi, b2lo]))
    shared["wini"] = np.ascontiguousarray(
        np.vstack([np.asarray(W_init, f32), np.asarray(b_init, f32)[None, :]]) / dt
    )
    # bg broadcast per (partition, m-chunk) feature-major for the psC fold
    shared["bgb"] = np.ascontiguousarray(
        np.broadcast_to(
            np.asarray(bg, f32).reshape(KC, 128).T[:, :, None], (128, KC, BL)
        ).reshape(128, KC * BL)
    )
    shared["wh1"] = lhsT_layout(dt * np.asarray(Wh1, f32))
    shared["wh2"] = np.ascontiguousarray(
        np.asarray(Wh2, f32).reshape(KC, 128, O).transpose(1, 0, 2).reshape(128, KC * O)
    )
    shared["bh1t"] = np.ascontiguousarray(np.asarray(bh1, f32).reshape(KC, 128).T)
    shared["bh2t"] = np.asarray(bh2, f32).reshape(O, 1)

    coeffs = np.asarray(coeffs, f32)  # [B, T, C]
    dW = np.asarray(dW, f32)  # [NT_full, B, H]
    dw_scale = f32(0.5 * sq / dt)
    in_maps = []
    NTP = T
    for c in range(NCORES):
        bs = slice(c * BL, (c + 1) * BL)
        xt = np.empty((T, C + 1, BL), f32)
        xt[:, :C, :] = coeffs[bs].transpose(1, 2, 0)
        xt[:, C, :] = 1.0
        # packed [hi|lo] per t, feature-major, zero pad to NTP slots
        xhl = np.zeros((C + 1, NTP, 2, BL), f32)
        xhi, xlo = _split(xt[:NT].transpose(1, 0, 2))  # [C+1, NT, BL] each
        xhl[:, :NT, 0, :] = xhi.astype(f32)
        xhl[:, :NT, 1, :] = xlo.astype(f32)
        xhl = xhl.reshape(C + 1, NTP * 2 * BL)
        dwc = (dW[:NT, bs, :] * dw_scale).transpose(0, 2, 1)  # [NT, H, BL]
        dwc = np.ascontiguousarray(
            dwc.reshape(NT, KC, 128, BL).transpose(0, 2, 1, 3).reshape(NT, 128, KC * BL)
        )
        in_maps.append(
            {"xhl": np.asarray(xhl, BF), "x0": np.ascontiguousarray(xt[0]),
             "dw": dwc, **shared}
        )
    return in_maps


def kernel(**inputs):
    global _BUILT
    if _BUILT is None:
        _BUILT = _build_nc()
    nc = _BUILT
    in_maps = _prep_inputs(**inputs)
    res = run_bass_kernel_spmd(nc, in_maps, core_ids=list(range(NCORES)))
    out = np.empty((B, OUT_TIME, O), np.float32)
    for c, r in enumerate(res.results):
        out[c * BL : (c + 1) * BL] = (
            r["out"].reshape(O, OUT_TIME, BL).transpose(2, 1, 0)
        )
    return out
